# revision 5
# baseline (speedup 1.0000x reference)
"""Trainium2 Bass kernel: LiDAR point cloud -> range image (nn_Coordinate).

Full inputs in, full outputs out. Internally: shard the 16384 points across
8 NeuronCores (2048 each); each core computes per-point depth/weight/angles,
nearest grid cell via separable index arithmetic, scatters num/den grids via
a one-hot TensorE matmul, then an AllReduce(sum) combines the grids and every
core finishes the division/normalization.
"""

import os
import numpy as np

H, W = 64, 256
N_POINTS = 16384
NCORES = 8
PT = 128
NPC = N_POINTS // NCORES   # 2048 points per core
NF = NPC // PT             # 16 partition tiles per core

MAGIC = 12582912.0         # 1.5 * 2^23: (x + M) - M == round-to-nearest-int (RNE)
TAU = 2.0
MIN_DEPTH = 1.45
MAX_DEPTH = 80.0
HOST_REDUCE = bool(int(os.environ.get("BASSK_HOST_REDUCE", "0")))
TRACE = bool(int(os.environ.get("BASSK_TRACE", "0")))

LAST_EXEC_NS = None
LAST_RESULTS = None


def _atan_coeffs(n=12):
    # atan(t) ~= t * P(t^2) on t in [0,1]; max abs err ~4e-11 for n=12
    m = 4000
    k = np.arange(m)
    s = 0.5 - 0.5 * np.cos(np.pi * (k + 0.5) / m)
    t = np.sqrt(s)
    f = np.arctan(t) / t
    V = np.polynomial.chebyshev.chebvander(2 * s - 1, n - 1)
    c, *_ = np.linalg.lstsq(V, f, rcond=None)
    p = np.polynomial.chebyshev.Chebyshev(c, domain=[0, 1]).convert(
        kind=np.polynomial.Polynomial
    )
    return [float(x) for x in p.coef]


_CACHE = {}


def _build(e0, e63, z0, z255):
    import concourse.bacc as bacc
    import concourse.bass as bass
    import concourse.mybir as mybir
    import concourse.tile as tile

    f32 = mybir.dt.float32
    f16 = mybir.dt.float16
    i32 = mybir.dt.int32
    OP = mybir.AluOpType
    ACT = mybir.ActivationFunctionType

    inv_de = float((H - 1) / (e0 - e63))
    inv_dz = float((W - 1) / (z0 - z255))
    C = _atan_coeffs(12)
    PI = float(np.float32(np.pi))
    PIH = float(np.float32(np.pi / 2))

    nc = bacc.Bacc(None, target_bir_lowering=False)
    xyz_in = nc.dram_tensor("xyz", [NPC, 3], f32, kind="ExternalInput")
    if HOST_REDUCE:
        grid_out = nc.dram_tensor("grid", [2 * H, W], f32, kind="ExternalOutput")
    else:
        depth_out = nc.dram_tensor("depth", [H, W], f32, kind="ExternalOutput")
        valid_out = nc.dram_tensor("valid", [H, W], f32, kind="ExternalOutput")

    with tile.TileContext(nc) as tc:
        with tc.tile_pool(name="per", bufs=1) as per, \
             tc.tile_pool(name="lp", bufs=3) as lp, \
             tc.tile_pool(name="pp", bufs=1, space="PSUM") as pp, \
             tc.tile_pool(name="dr", bufs=1, space="DRAM") as dr:

            def tt(out, in0, in1, op):
                nc.vector.tensor_tensor(out=out, in0=in0, in1=in1, op=op)

            def ts(out, in0, s1, s2, op0, op1=None):
                if s2 is None:
                    nc.vector.tensor_scalar(
                        out=out, in0=in0, scalar1=s1, scalar2=None, op0=op0
                    )
                else:
                    nc.vector.tensor_scalar(
                        out=out, in0=in0, scalar1=s1, scalar2=s2, op0=op0, op1=op1
                    )

            # ---- constants ----
            iota_i = per.tile([PT, W], i32)
            nc.gpsimd.iota(iota_i[:, :], pattern=[[1, W]], base=0, channel_multiplier=0)
            iota_f = per.tile([PT, W], f32)
            nc.vector.tensor_copy(out=iota_f[:, :], in_=iota_i[:, :])

            # ---- load xyz shard (contiguous), split into X/Y/Z [PT, NF] ----
            xyzt = per.tile([PT, 3 * NF], f32)
            nc.sync.dma_start(
                out=xyzt[:, :], in_=bass.AP(xyz_in, 0, [[3 * NF, PT], [1, 3 * NF]])
            )
            X = per.tile([PT, NF], f32)
            Y = per.tile([PT, NF], f32)
            Z = per.tile([PT, NF], f32)
            for off, dst in ((0, X), (1, Y), (2, Z)):
                nc.vector.tensor_copy(
                    out=dst[:, :],
                    in_=bass.AP(xyzt.tensor, off, [[3 * NF, PT], [3, NF]]),
                )

            # ---- pointwise stage ([PT, NF] tiles, fp32) ----
            x2 = per.tile([PT, NF], f32)
            y2 = per.tile([PT, NF], f32)
            z2 = per.tile([PT, NF], f32)
            r2 = per.tile([PT, NF], f32)
            dd2 = per.tile([PT, NF], f32)
            tt(x2[:, :], X[:, :], X[:, :], OP.mult)
            tt(y2[:, :], Y[:, :], Y[:, :], OP.mult)
            tt(z2[:, :], Z[:, :], Z[:, :], OP.mult)
            tt(r2[:, :], x2[:, :], y2[:, :], OP.add)
            tt(dd2[:, :], r2[:, :], z2[:, :], OP.add)
            r = per.tile([PT, NF], f32)
            depth = per.tile([PT, NF], f32)
            nc.scalar.activation(out=r[:, :], in_=r2[:, :], func=ACT.Sqrt)
            nc.scalar.activation(out=depth[:, :], in_=dd2[:, :], func=ACT.Sqrt)
            w0 = per.tile([PT, NF], f32)
            nc.scalar.activation(out=w0[:, :], in_=depth[:, :], func=ACT.Exp, scale=-TAU)
            d80 = per.tile([PT, NF], f32)
            ts(d80[:, :], depth[:, :], MAX_DEPTH, None, OP.mult)
            wma = per.tile([PT, NF], f32)
            wgt = per.tile([PT, NF], f32)
            nc.vector.scalar_tensor_tensor(
                out=wma[:, :], in0=d80[:, :], scalar=MIN_DEPTH, in1=w0[:, :],
                op0=OP.is_gt, op1=OP.mult,
            )
            nc.vector.scalar_tensor_tensor(
                out=wgt[:, :], in0=d80[:, :], scalar=MAX_DEPTH, in1=wma[:, :],
                op0=OP.is_lt, op1=OP.mult,
            )
            val = per.tile([PT, NF], f32)
            tt(val[:, :], wgt[:, :], d80[:, :], OP.mult)

            # ---- atan helper ----
            def emit_atan(out_t, t_in, tag):
                a = lp.tile([PT, NF], f32, name=f"at_a_{tag}", tag="at_a")
                inv = lp.tile([PT, NF], f32, name=f"at_inv_{tag}", tag="at_inv")
                tm = lp.tile([PT, NF], f32, name=f"at_tm_{tag}", tag="at_tm")
                s = lp.tile([PT, NF], f32, name=f"at_s_{tag}", tag="at_s")
                p = lp.tile([PT, NF], f32, name=f"at_p_{tag}", tag="at_p")
                pm = lp.tile([PT, NF], f32, name=f"at_pm_{tag}", tag="at_pm")
                big = lp.tile([PT, NF], f32, name=f"at_big_{tag}", tag="at_big")
                fx = lp.tile([PT, NF], f32, name=f"at_fx_{tag}", tag="at_fx")
                sg = lp.tile([PT, NF], f32, name=f"at_sg_{tag}", tag="at_sg")
                nc.scalar.activation(out=a[:, :], in_=t_in[:, :], func=ACT.Abs)
                nc.vector.reciprocal(out=inv[:, :], in_=a[:, :])
                tt(tm[:, :], a[:, :], inv[:, :], OP.min)
                tt(s[:, :], tm[:, :], tm[:, :], OP.mult)
                ts(p[:, :], s[:, :], C[11], C[10], OP.mult, OP.add)
                for k in range(9, -1, -1):
                    tt(pm[:, :], p[:, :], s[:, :], OP.mult)
                    ts(p[:, :], pm[:, :], C[k], None, OP.add)
                tt(p[:, :], tm[:, :], p[:, :], OP.mult)  # p = atan(min(|t|,1/|t|))
                ts(big[:, :], a[:, :], 1.0, None, OP.is_gt)
                ts(fx[:, :], p[:, :], -2.0, PIH, OP.mult, OP.add)
                tt(fx[:, :], fx[:, :], big[:, :], OP.mult)
                tt(p[:, :], p[:, :], fx[:, :], OP.add)
                ts(sg[:, :], t_in[:, :], 0.0, None, OP.is_ge)
                ts(sg[:, :], sg[:, :], 2.0, -1.0, OP.mult, OP.add)
                tt(out_t[:, :], p[:, :], sg[:, :], OP.mult)

            invr = per.tile([PT, NF], f32)
            nc.vector.reciprocal(out=invr[:, :], in_=r[:, :])
            tu = per.tile([PT, NF], f32)
            tt(tu[:, :], Z[:, :], invr[:, :], OP.mult)
            u = per.tile([PT, NF], f32)
            emit_atan(u, tu, "u")

            invx = per.tile([PT, NF], f32)
            nc.vector.reciprocal(out=invx[:, :], in_=X[:, :])
            q = per.tile([PT, NF], f32)
            tt(q[:, :], Y[:, :], invx[:, :], OP.mult)
            v = per.tile([PT, NF], f32)
            emit_atan(v, q, "v")
            # v += pi * sign(y) * (x<0)
            sgy = per.tile([PT, NF], f32)
            xneg = per.tile([PT, NF], f32)
            ts(sgy[:, :], Y[:, :], 0.0, None, OP.is_ge)
            ts(sgy[:, :], sgy[:, :], 2.0, -1.0, OP.mult, OP.add)
            ts(xneg[:, :], X[:, :], 0.0, None, OP.is_lt)
            tt(xneg[:, :], xneg[:, :], sgy[:, :], OP.mult)
            nc.vector.scalar_tensor_tensor(
                out=v[:, :], in0=xneg[:, :], scalar=PI, in1=v[:, :],
                op0=OP.mult, op1=OP.add,
            )

            # ---- grid indices: i = clamp(rne((e0-u)*inv_de)), j likewise ----
            ii = per.tile([PT, NF], f32)
            jj = per.tile([PT, NF], f32)
            ts(ii[:, :], u[:, :], -inv_de, float(e0) * inv_de, OP.mult, OP.add)
            ts(ii[:, :], ii[:, :], MAGIC, MAGIC, OP.add, OP.subtract)
            ts(ii[:, :], ii[:, :], 0.0, float(H - 1), OP.max, OP.min)
            ts(jj[:, :], v[:, :], -inv_dz, float(z0) * inv_dz, OP.mult, OP.add)
            ts(jj[:, :], jj[:, :], MAGIC, MAGIC, OP.add, OP.subtract)
            ts(jj[:, :], jj[:, :], 0.0, float(W - 1), OP.max, OP.min)

            # ---- scatter: psum[0:64]=den, psum[64:128]=num ----
            psum = pp.tile([PT, W], f32)
            for f in range(NF):
                icol = ii[:, f:f + 1]
                jcol = jj[:, f:f + 1]
                wcol = wgt[:, f:f + 1]
                vcol = val[:, f:f + 1]
                dj = lp.tile([PT, W], f32, name=f"dj_{f}", tag="dj")
                ts(dj[:, :], iota_f[:, :], jcol, None, OP.subtract)
                tt(dj[:, :], dj[:, :], dj[:, :], OP.mult)
                ohj = lp.tile([PT, W], f16, name=f"ohj_{f}", tag="ohj")
                nc.scalar.activation(
                    out=ohj[:, :], in_=dj[:, :], func=ACT.Relu, bias=1.0, scale=-1.0
                )
                di = lp.tile([PT, H], f32, name=f"di_{f}", tag="di")
                ts(di[:, :], iota_f[:, 0:H], icol, None, OP.subtract)
                tt(di[:, :], di[:, :], di[:, :], OP.mult)
                ohi = lp.tile([PT, H], f32, name=f"ohi_{f}", tag="ohi")
                nc.scalar.activation(
                    out=ohi[:, :], in_=di[:, :], func=ACT.Relu, bias=1.0, scale=-1.0
                )
                lhsT = lp.tile([PT, 2 * H], f16, name=f"lhsT_{f}", tag="lhsT")
                ts(lhsT[:, 0:H], ohi[:, :], wcol, None, OP.mult)
                ts(lhsT[:, H:2 * H], ohi[:, :], vcol, None, OP.mult)
                nc.tensor.matmul(
                    out=psum[:, :], lhsT=lhsT[:, :], rhs=ohj[:, :],
                    start=(f == 0), stop=(f == NF - 1),
                )

            grid = per.tile([PT, W], f32)
            nc.vector.tensor_copy(out=grid[:, :], in_=psum[:, :])

            if HOST_REDUCE:
                nc.sync.dma_start(out=grid_out[:, :], in_=grid[:, :])
            else:
                inb = dr.tile([PT, W], f32)
                outb = dr.tile([PT, W], f32)
                nc.sync.dma_start(out=inb[:, :], in_=grid[:, :])
                nc.gpsimd.collective_compute(
                    "AllReduce",
                    OP.add,
                    replica_groups=[list(range(NCORES))],
                    ins=[inb[:, :].opt()],
                    outs=[outb[:, :].opt()],
                )
                red = per.tile([H, 2 * W], f32)
                nc.sync.dma_start(out=red[:, 0:W], in_=outb[0:H, :])
                nc.sync.dma_start(out=red[:, W:2 * W], in_=outb[H:2 * H, :])

                den = red[:, 0:W]
                num = red[:, W:2 * W]
                d1 = per.tile([H, W], f32)
                ts(d1[:, :], den, 1e-8, None, OP.add)
                rc = per.tile([H, W], f32)
                nc.vector.reciprocal(out=rc[:, :], in_=d1[:, :])
                dep = per.tile([H, W], f32)
                tt(dep[:, :], num, rc[:, :], OP.mult)
                vld = per.tile([H, W], f32)
                ts(vld[:, :], dep[:, :], 0.0, None, OP.is_gt)
                sc = 1.0 / (MAX_DEPTH - MIN_DEPTH)
                t1 = per.tile([H, W], f32)
                ts(t1[:, :], dep[:, :], sc, -MIN_DEPTH * sc, OP.mult, OP.add)
                tt(t1[:, :], t1[:, :], vld[:, :], OP.mult)
                nv = per.tile([H, W], f32)
                ts(nv[:, :], vld[:, :], -1.0, 1.0, OP.mult, OP.add)
                outd = per.tile([H, W], f32)
                tt(outd[:, :], t1[:, :], nv[:, :], OP.add)
                nc.sync.dma_start(out=depth_out[:, :], in_=outd[:, :])
                nc.sync.dma_start(out=valid_out[:, :], in_=vld[:, :])

    nc.compile()
    return nc


def _install_ntff_shim():
    # The image's antenv package lacks axon_hooks; recreate the hook from
    # trn_agent_boot's ctypes implementation so trace=True yields NTFF
    # profiles. No-op when unavailable (tracing then degrades gracefully).
    import sys
    import types
    if "antenv.axon_hooks" in sys.modules:
        return
    try:
        from antenv import axon_hooks  # noqa: F401
        return
    except ImportError:
        pass
    try:
        sys.path.insert(0, "/root/.axon_site")
        from trn_agent_boot.trn_boot import _ntff_profile_via_ctypes
        hook = _ntff_profile_via_ctypes("/opt/axon/libaxon_pjrt.so")
        mod = types.ModuleType("antenv.axon_hooks")
        mod.get_axon_ntff_profile_hook = lambda: hook
        mod.set_axon_ntff_profile_hook = lambda h: None
        sys.modules["antenv.axon_hooks"] = mod
    except Exception:
        pass


def kernel(xyz, angle):
    global LAST_EXEC_NS, LAST_RESULTS
    from concourse.bass_utils import run_bass_kernel_spmd

    if TRACE:
        _install_ntff_shim()

    xyz = np.asarray(xyz, dtype=np.float32)
    angle = np.asarray(angle, dtype=np.float64)
    e0 = float(angle[0, 0, 0, 0])
    e63 = float(angle[0, 0, H - 1, 0])
    z0 = float(angle[0, 1, 0, 0])
    z255 = float(angle[0, 1, 0, W - 1])

    key = (e0, e63, z0, z255, HOST_REDUCE)
    if key not in _CACHE:
        _CACHE[key] = _build(e0, e63, z0, z255)
    nc = _CACHE[key]

    pts = xyz.reshape(N_POINTS, 3)
    in_maps = [
        {"xyz": np.ascontiguousarray(pts[c * NPC:(c + 1) * NPC])}
        for c in range(NCORES)
    ]
    res = run_bass_kernel_spmd(nc, in_maps, core_ids=list(range(NCORES)), trace=TRACE)
    LAST_EXEC_NS = res.exec_time_ns
    LAST_RESULTS = res

    if HOST_REDUCE:
        g = np.zeros((2 * H, W), np.float32)
        for c in range(NCORES):
            g += np.asarray(res.results[c]["grid"], np.float32)
        den = g[0:H]
        num = g[H:2 * H]
        dep = (num / (den + np.float32(1e-8))).astype(np.float32)
        valid = dep != 0
        sc = np.float32(1.0 / (MAX_DEPTH - MIN_DEPTH))
        out = np.where(valid, (dep - np.float32(MIN_DEPTH)) * sc, np.float32(1.0))
        return out[None, None].astype(np.float32), valid[None, None]

    dep = np.asarray(res.results[0]["depth"], np.float32).reshape(1, 1, H, W)
    vld = np.asarray(res.results[0]["valid"], np.float32) != 0
    return dep, vld.reshape(1, 1, H, W)
